# revision 2
# baseline (speedup 1.0000x reference)
"""Trainium2 Bass kernel for nn_Graph_to_Featuremaps_savemem.

Math: the reference computes, per batch b,
    scores[b,p,n] = (res @ nfr)[b,p] + (x @ nfh)[b,n]
    attn = softmax_n(scores);  out[b,p,c] = (attn @ (x @ W))[b,p,c]
Softmax over n is shift-invariant, so the (res @ nfr)[b,p] term cancels:
    attn[b,p,:] = softmax(x[b] @ nfh)   (independent of p)
    out[b,c,h,w] = relu(((softmax(x[b]@nfh) @ x[b]) @ W)[c])   broadcast over (h,w)
res_feature never affects the output. The kernel is a tiny per-batch compute
(one 64-softmax + two small matmuls) followed by a 256 MB broadcast write --
pure HBM-write-bound, sharded batch-parallel over 8 cores (2 batches, 32 MB
written per core).

Device-side chain (all tiny; X is passed pre-transposed so no PE transpose):
  s  = X @ nfh            one matmul (lhsT = XT)
  e  = exp(s)             (softmax shift skipped; scores are O(1))
  E  = MASK * e           (128,2): column b holds e masked to batch b's nodes
  M  = X @ W              one matmul (lhsT = XT), copied to SBUF
  S  = ONES128^T @ E      (128,2): per-batch sum broadcast to all partitions
  RC = 1/S                (128,2)
  V_c = M[:,c]^T @ E      (128,2) per c-half: unnormalized output columns
  fill[b,c] = (0 max V_c[:,b]) * RC[:,b]   broadcast to (128, FILL_F) on DVE
Output: per (b, c-half) one 8 MB fully-contiguous DMA whose source AP re-reads
the (128, FILL_F) fill tile via a stride-0 middle dim, alternating the two
HWDGE rings (SP / ACT).
"""

import numpy as np

N_CORES = 8
B, NODES, HID, C, H, W = 16, 64, 128, 256, 128, 128
HWP = H * W  # 16384
B_LOC = B // N_CORES  # 2 batches per core
BN = B_LOC * NODES  # 128
FILL_F = 2048  # free-dim width of the broadcast fill tiles in SBUF

_NC_CACHE = {}


def build_nc():
    import concourse.bass as bass
    import concourse.bacc as bacc
    import concourse.mybir as mybir
    from concourse.tile import TileContext

    f32 = mybir.dt.float32
    Alu = mybir.AluOpType
    Act = mybir.ActivationFunctionType

    nc = bacc.Bacc(None, target_bir_lowering=False, debug=False)
    xt_d = nc.declare_dram_parameter("xt", [HID, BN], f32, isOutput=False)
    nfh_d = nc.declare_dram_parameter("nfh", [HID, 1], f32, isOutput=False)
    w_d = nc.declare_dram_parameter("w", [HID, C], f32, isOutput=False)
    out_d = nc.declare_dram_parameter("out", [B_LOC * C, HWP], f32, isOutput=True)

    def bcast_mid(ap, n):
        # (P,F) AP -> (P,n,F) AP re-reading the same F elements n times
        return type(ap)(ap.tensor, ap.offset, [list(ap.ap[0]), [0, n], list(ap.ap[1])])

    with TileContext(nc) as tc:
        with (
            tc.tile_pool(name="singles", bufs=1) as singles,
            tc.tile_pool(name="psum", bufs=1, space="PSUM") as psum,
        ):
            # ---- inputs first so their DMAs issue as early as possible ----
            XT = singles.tile([HID, BN], f32, tag="XT")
            nc.sync.dma_start(out=XT[:], in_=xt_d[:])
            NFH = singles.tile([HID, 1], f32, tag="NFH")
            nc.sync.dma_start(out=NFH[:], in_=nfh_d[:])
            Wt = singles.tile([HID, C], f32, tag="Wt")
            nc.scalar.dma_start(out=Wt[:], in_=w_d[:])

            # ---- constants (no input deps) ----
            ONES128 = singles.tile([128, 128], f32, tag="ONES128")
            nc.vector.memset(ONES128[:], 1.0)
            MASK = singles.tile([128, B_LOC], f32, tag="MASK")
            nc.vector.memset(MASK[:], 0.0)
            for b in range(B_LOC):
                nc.vector.memset(MASK[b * NODES : (b + 1) * NODES, b : b + 1], 1.0)
            ZERO = singles.tile([128, FILL_F], f32, tag="ZERO")
            nc.vector.memset(ZERO[:], 0.0)

            # ---- s = X @ nfh ; e = exp(s) ; E = MASK * e ----
            s_ps = psum.tile([BN, 1], f32, tag="s")
            nc.tensor.matmul(s_ps[:], XT[:], NFH[:])
            e_col = singles.tile([BN, 1], f32, tag="e_col")
            nc.scalar.activation(e_col[:], s_ps[:], Act.Exp)
            E = singles.tile([128, B_LOC], f32, tag="E")
            nc.vector.tensor_scalar(E[:], MASK[:], e_col[:], None, op0=Alu.mult)

            # ---- M = X @ W (independent of the e-chain) ----
            M_ps = psum.tile([BN, C], f32, tag="M")
            nc.tensor.matmul(M_ps[:], XT[:], Wt[:])
            M_sb = singles.tile([BN, C], f32, tag="M_sb")
            nc.vector.tensor_copy(M_sb[:], M_ps[:])

            # ---- per-batch sums broadcast to all partitions; RC = 1/S ----
            S_ps = psum.tile([128, B_LOC], f32, tag="S")
            nc.tensor.matmul(S_ps[:], ONES128[:], E[:])
            RC = singles.tile([128, B_LOC], f32, tag="RC")
            nc.vector.reciprocal(RC[:], S_ps[:])

            # ---- V_c = M[:,c]^T @ E : (128, 2) per c-half ----
            V_ps = []
            for hf in range(C // 128):
                vp = psum.tile([128, B_LOC], f32, tag=f"V{hf}")
                nc.tensor.matmul(vp[:], M_sb[:, hf * 128 : (hf + 1) * 128], E[:])
                V_ps.append(vp)

            # ---- fills + one 8 MB contiguous DMA per (batch, c-half) ----
            for b in range(B_LOC):
                for hf in range(C // 128):
                    fill = singles.tile([128, FILL_F], f32, tag=f"fill{b}{hf}")
                    # fill[p,f] = max(0, V[p,b]) * RC[p,b]  (== relu(V/S))
                    nc.vector.tensor_scalar(
                        fill[:],
                        ZERO[:],
                        V_ps[hf][:, b : b + 1],
                        RC[:, b : b + 1],
                        op0=Alu.max,
                        op1=Alu.mult,
                    )
                    r0 = b * C + hf * 128
                    eng = nc.sync if (b * 2 + hf) % 2 == 0 else nc.scalar
                    eng.dma_start(
                        out=out_d[r0 : r0 + 128, :],
                        in_=bcast_mid(fill[:], HWP // FILL_F),
                    )
    nc.finalize()
    return nc


def get_nc():
    if "nc" not in _NC_CACHE:
        _NC_CACHE["nc"] = build_nc()
    return _NC_CACHE["nc"]


def make_in_maps(input, node_fea_for_hidden, weight):
    x = np.asarray(input, np.float32)[0]  # (B, NODES, HID)
    nfh = np.ascontiguousarray(np.asarray(node_fea_for_hidden, np.float32).reshape(HID, 1))
    w = np.ascontiguousarray(np.asarray(weight, np.float32))
    in_maps = []
    for i in range(N_CORES):
        xt = np.ascontiguousarray(
            x[i * B_LOC : (i + 1) * B_LOC].reshape(BN, HID).T
        )
        in_maps.append({"xt": xt, "nfh": nfh, "w": w})
    return in_maps


def run_spmd(in_maps, trace=False, **kw):
    from concourse.bass_utils import run_bass_kernel_spmd

    return run_bass_kernel_spmd(get_nc(), in_maps, list(range(N_CORES)), trace=trace, **kw)


def kernel(input, res_feature, node_fea_for_res, node_fea_for_hidden, weight):
    res = run_spmd(make_in_maps(input, node_fea_for_hidden, weight)).results
    out = np.concatenate(
        [r["out"].reshape(B_LOC, C, H, W) for r in res], axis=0
    )
    return out
